# revision 18
# baseline (speedup 1.0000x reference)
"""Qlinear (dequant int8 matmul + bias) Trainium2 kernel.

Reference computation (zero points subtracted exactly; per-output-channel
scales):
    out[m, n] = (sum_k (inp[m,k]-izp) * (w[n,k]-wzp[n])) * (is * ws[n]) + bias[n]

Strategy: int8-range values are exactly representable in bf16, and the PE
accumulates bf16 matmuls into fp32 PSUM, so the integer matmul is exact on
the TensorEngine.  The per-channel scale+bias is applied on PSUM eviction
with N on the partition axis (per-partition scale/bias on the ScalarEngine).

Sharding: M (rows of inp) is sharded across the 8 cores; each core keeps its
x-shard resident in SBUF (8MB bf16) and streams the full transposed weight
(33.5MB bf16) through SBUF in [K, 512] tiles.

Host pre-tiles both operands into the SBUF layout ([partition, ...tile dims])
so every DMA moves 8-32KB contiguous runs per partition (full HBM bandwidth);
the load order matches consumption order so the PE starts ~16us in and
DMA-chases the head instead of waiting for the full 12MB input fill.
"""

import numpy as np
import ml_dtypes

M, K, N = 8192, 4096, 4096
NCORES = 8
ML = M // NCORES  # 1024 rows of inp per core
P = 128
KT = K // P  # 32 k-tiles
FD = 512  # matmul moving free dim / PSUM bank width (fp32)
NTO = N // FD  # 8 outer n-tiles (one wsb load each)
NTI = FD // P  # 4 inner n-tiles (psum partition dim)
MH = ML // FD  # 2 psum groups over local M

_CACHE: dict = {}


def build_nc(ml=ML, k=K, n=N):
    import concourse.mybir as mybir
    import concourse.tile as tile
    from concourse import bacc

    kt = k // P
    nto = n // FD
    nti = FD // P
    mh = ml // FD

    nc = bacc.Bacc(None, target_bir_lowering=False)
    xt = nc.dram_tensor("xt", [P, mh, kt, FD], mybir.dt.bfloat16, kind="ExternalInput")
    wt = nc.dram_tensor(
        "wt", [P, nto, nti, kt, P], mybir.dt.bfloat16, kind="ExternalInput"
    )
    sclbia = nc.dram_tensor("sclbia", [2, n], mybir.dt.float32, kind="ExternalInput")
    outT = nc.dram_tensor("outT", [n, ml], mybir.dt.float32, kind="ExternalOutput")

    sclbia3 = sclbia.rearrange("t (nt p) -> p t nt", p=P)
    outT3 = outT.rearrange("(nt p) m -> p nt m", p=P)

    with tile.TileContext(nc) as tc:
        with (
            tc.tile_pool(name="const", bufs=1) as const,
            tc.tile_pool(name="xpool", bufs=1) as xpool,
            tc.tile_pool(name="wpool", bufs=2) as wpool,
            tc.tile_pool(name="opool", bufs=4) as opool,
            tc.tile_pool(name="psum", bufs=8, space="PSUM") as pspool,
        ):
            # Load order = consumption order: w[no=0,ni=0], x[mc=0], x[mc=1],
            # rest of w[no=0], consts, then w[no] streamed per outer n-tile.
            wsb0 = wpool.tile([P, nti, kt, P], mybir.dt.bfloat16, tag="wsb")
            nc.sync.dma_start(wsb0[:, 0], wt[:, 0, 0])
            xsb = xpool.tile([P, mh, kt, FD], mybir.dt.bfloat16)
            for mc in range(mh):
                nc.sync.dma_start(xsb[:, mc], xt[:, mc])
            nc.sync.dma_start(wsb0[:, 1:], wt[:, 0, 1:])

            # Stage the per-channel scale/bias, then copy via the ScalarE so
            # the per-tile evictions (also on ScalarE) depend on it through
            # engine program order (engine instructions carry few sync waits).
            csb_stage = const.tile([P, 2, n // P], mybir.dt.float32)
            nc.sync.dma_start(csb_stage[:], sclbia3[:])
            csb = const.tile([P, 2, n // P], mybir.dt.float32)
            nc.scalar.copy(csb[:], csb_stage[:])
            scl_sb = csb[:, 0]
            bia_sb = csb[:, 1]

            for no in range(nto):
                if no == 0:
                    wsb = wsb0
                else:
                    wsb = wpool.tile([P, nti, kt, P], mybir.dt.bfloat16, tag="wsb")
                    nc.sync.dma_start(wsb[:], wt[:, no])
                for ni in range(nti):
                    n_abs = no * nti + ni
                    for mc in range(mh):
                        ps = pspool.tile([P, FD], mybir.dt.float32)
                        for ko in range(kt):
                            nc.tensor.matmul(
                                ps[:],
                                wsb[:, ni, ko, :],
                                xsb[:, mc, ko, :],
                                start=(ko == 0),
                                stop=(ko == kt - 1),
                            )
                        osb = opool.tile([P, FD], mybir.dt.float32)
                        nc.scalar.activation(
                            osb[:],
                            ps[:],
                            mybir.ActivationFunctionType.Identity,
                            bias=bia_sb[:, n_abs : n_abs + 1],
                            scale=scl_sb[:, n_abs : n_abs + 1],
                        )
                        nc.sync.dma_start(
                            outT3[:, n_abs, mc * FD : (mc + 1) * FD], osb[:]
                        )
    nc.compile()
    return nc


def _get_nc():
    if "nc" not in _CACHE:
        _CACHE["nc"] = build_nc()
    return _CACHE["nc"]


def _prep_in_maps(inputs):
    bf16 = ml_dtypes.bfloat16
    inp = np.asarray(inputs["inp"])
    weight = np.asarray(inputs["weight"])
    bias = np.ascontiguousarray(np.asarray(inputs["bias"], dtype=np.float32))
    s_in = np.asarray(inputs["inp_scales"], dtype=np.float32)
    zp_in = np.asarray(inputs["inp_zero_points"])
    s_w = np.asarray(inputs["weight_scales"], dtype=np.float32)
    zp_w = np.asarray(inputs["weight_zero_points"])

    x_eff = (inp.astype(np.int32) - np.int32(zp_in.reshape(-1)[0])).astype(bf16)
    w_eff = (weight.astype(np.int32) - zp_w.reshape(-1)[:, None].astype(np.int32)).astype(bf16)

    # w_tiled[p, no, ni, ko, nn] = w_eff[no*FD + ni*P + nn, ko*P + p]
    w_tiled = np.ascontiguousarray(
        w_eff.reshape(NTO, NTI, P, KT, P).transpose(4, 0, 1, 3, 2)
    )
    scl = (s_in.reshape(-1)[0] * s_w).astype(np.float32)
    sclbia = np.ascontiguousarray(np.stack([scl, bias]))  # [2, N]

    in_maps = []
    for c in range(NCORES):
        # x_tiled[p, mc, ko, mm] = x_eff[c*ML + mc*FD + mm, ko*P + p]
        xc = x_eff[c * ML : (c + 1) * ML]
        x_tiled = np.ascontiguousarray(
            xc.reshape(MH, FD, KT, P).transpose(3, 0, 2, 1)
        )
        in_maps.append({"xt": x_tiled, "wt": w_tiled, "sclbia": sclbia})
    return in_maps


def _run(inputs, trace=False):
    from concourse.bass_utils import run_bass_kernel_spmd

    nc = _get_nc()
    in_maps = _prep_in_maps(inputs)
    res = run_bass_kernel_spmd(nc, in_maps, list(range(NCORES)), trace=trace)
    out = np.empty((M, N), dtype=np.float32)
    for c in range(NCORES):
        out[c * ML : (c + 1) * ML, :] = res.results[c]["outT"].T
    return out, res


def _run_in_subprocess(inputs) -> np.ndarray:
    """Re-run in a fresh process. The axon/NRT first execution occasionally
    dies with NRT_EXEC_UNIT_UNRECOVERABLE and the in-process jax client is
    not reliably usable afterwards; a fresh process recovers."""
    import os
    import subprocess
    import sys
    import tempfile

    tmp = tempfile.mkdtemp(prefix="qlin_")
    in_path = os.path.join(tmp, "in.npz")
    out_path = os.path.join(tmp, "out.npy")
    np.savez(in_path, **{k: np.asarray(v) for k, v in inputs.items()})
    code = (
        "import importlib.util, numpy as np\n"
        f"spec = importlib.util.spec_from_file_location('qlin_kernel', {os.path.abspath(__file__)!r})\n"
        "m = importlib.util.module_from_spec(spec)\n"
        "spec.loader.exec_module(m)\n"
        f"data = np.load({in_path!r})\n"
        "inputs = {k: data[k] for k in data.files}\n"
        "out, _ = m._run(inputs, trace=False)\n"
        f"np.save({out_path!r}, out)\n"
    )
    env = dict(os.environ, QLIN_NO_SUBPROC="1")
    last_err = None
    for _ in range(2):
        try:
            subprocess.run(
                [sys.executable, "-c", code], env=env, check=True, timeout=1800
            )
            return np.load(out_path)
        except Exception as e:  # noqa: BLE001 - retry then re-raise
            last_err = e
    raise last_err


def kernel(**inputs) -> np.ndarray:
    import os

    try:
        out, _ = _run(inputs, trace=False)
        return out
    except Exception:
        if os.environ.get("QLIN_NO_SUBPROC"):
            raise
    return _run_in_subprocess(inputs)


# revision 20
# speedup vs baseline: 1.0088x; 1.0088x over previous
"""Qlinear (dequant int8 matmul + bias) Trainium2 kernel.

Reference computation (zero points subtracted exactly; per-output-channel
scales):
    out[m, n] = (sum_k (inp[m,k]-izp) * (w[n,k]-wzp[n])) * (is * ws[n]) + bias[n]

Strategy: int8-range values are exactly representable in bf16, and the PE
accumulates bf16 matmuls into fp32 PSUM, so the integer matmul is exact on
the TensorEngine.  The per-channel scale+bias is applied on PSUM eviction
with N on the partition axis (per-partition scale/bias on the ScalarEngine).

Sharding: M (rows of inp) is sharded across the 8 cores; each core keeps its
x-shard resident in SBUF (8MB bf16) and streams the full transposed weight
(33.5MB bf16) through SBUF in [K, 512] tiles.

Host pre-tiles both operands into the SBUF layout ([partition, ...tile dims])
so every DMA moves 8-32KB contiguous runs per partition (full HBM bandwidth);
the load order matches consumption order so the PE starts ~16us in and
DMA-chases the head instead of waiting for the full 12MB input fill.
"""

import numpy as np
import ml_dtypes

M, K, N = 8192, 4096, 4096
NCORES = 8
ML = M // NCORES  # 1024 rows of inp per core
P = 128
KT = K // P  # 32 k-tiles
FD = 512  # matmul moving free dim / PSUM bank width (fp32)
NTO = N // FD  # 8 outer n-tiles (one wsb load each)
NTI = FD // P  # 4 inner n-tiles (psum partition dim)
MH = ML // FD  # 2 psum groups over local M

_CACHE: dict = {}


def build_nc(ml=ML, k=K, n=N):
    import concourse.mybir as mybir
    import concourse.tile as tile
    from concourse import bacc

    kt = k // P
    nto = n // FD
    nti = FD // P
    mh = ml // FD

    nc = bacc.Bacc(None, target_bir_lowering=False)
    xt = nc.dram_tensor("xt", [P, mh, kt, FD], mybir.dt.bfloat16, kind="ExternalInput")
    wt = nc.dram_tensor(
        "wt", [P, nto, nti, kt, P], mybir.dt.bfloat16, kind="ExternalInput"
    )
    sclbia = nc.dram_tensor("sclbia", [2, n], mybir.dt.float32, kind="ExternalInput")
    outT = nc.dram_tensor("outT", [n, ml], mybir.dt.float32, kind="ExternalOutput")

    sclbia3 = sclbia.rearrange("t (nt p) -> p t nt", p=P)
    outT3 = outT.rearrange("(nt p) m -> p nt m", p=P)

    with tile.TileContext(nc) as tc:
        with (
            tc.tile_pool(name="const", bufs=1) as const,
            tc.tile_pool(name="xpool", bufs=1) as xpool,
            tc.tile_pool(name="wpool", bufs=2) as wpool,
            tc.tile_pool(name="opool", bufs=4) as opool,
            tc.tile_pool(name="psum", bufs=8, space="PSUM") as pspool,
        ):
            # Load order = consumption order: w[no=0,ni=0], x[mc=0], x[mc=1],
            # rest of w[no=0], consts, then w[no] streamed per outer n-tile.
            wsb0 = wpool.tile([P, nti, kt, P], mybir.dt.bfloat16, tag="wsb")
            nc.sync.dma_start(wsb0[:, 0], wt[:, 0, 0])
            xsb = xpool.tile([P, mh, kt, FD], mybir.dt.bfloat16)
            kh = kt // 2
            nc.sync.dma_start(xsb[:, 0, :kh], xt[:, 0, :kh])
            nc.sync.dma_start(xsb[:, 0, kh:], xt[:, 0, kh:])
            for mc in range(1, mh):
                nc.sync.dma_start(xsb[:, mc], xt[:, mc])
            nc.sync.dma_start(wsb0[:, 1:], wt[:, 0, 1:])

            # Stage the per-channel scale/bias (not needed until the first
            # eviction), then copy via the ScalarE so the per-tile evictions
            # (also on ScalarE) depend on it through engine program order
            # (engine instructions carry few sync waits).
            csb_stage = const.tile([P, 2, n // P], mybir.dt.float32)
            nc.scalar.dma_start(csb_stage[:], sclbia3[:])
            csb = const.tile([P, 2, n // P], mybir.dt.float32)
            nc.scalar.copy(csb[:], csb_stage[:])
            scl_sb = csb[:, 0]
            bia_sb = csb[:, 1]

            for no in range(nto):
                if no == 0:
                    wsb = wsb0
                else:
                    wsb = wpool.tile([P, nti, kt, P], mybir.dt.bfloat16, tag="wsb")
                    nc.sync.dma_start(wsb[:], wt[:, no])
                for ni in range(nti):
                    n_abs = no * nti + ni
                    for mc in range(mh):
                        ps = pspool.tile([P, FD], mybir.dt.float32)
                        for ko in range(kt):
                            nc.tensor.matmul(
                                ps[:],
                                wsb[:, ni, ko, :],
                                xsb[:, mc, ko, :],
                                start=(ko == 0),
                                stop=(ko == kt - 1),
                            )
                        osb = opool.tile([P, FD], mybir.dt.float32)
                        nc.scalar.activation(
                            osb[:],
                            ps[:],
                            mybir.ActivationFunctionType.Identity,
                            bias=bia_sb[:, n_abs : n_abs + 1],
                            scale=scl_sb[:, n_abs : n_abs + 1],
                        )
                        nc.sync.dma_start(
                            outT3[:, n_abs, mc * FD : (mc + 1) * FD], osb[:]
                        )
    nc.compile()
    return nc


def _get_nc():
    if "nc" not in _CACHE:
        _CACHE["nc"] = build_nc()
    return _CACHE["nc"]


def _prep_in_maps(inputs):
    bf16 = ml_dtypes.bfloat16
    inp = np.asarray(inputs["inp"])
    weight = np.asarray(inputs["weight"])
    bias = np.ascontiguousarray(np.asarray(inputs["bias"], dtype=np.float32))
    s_in = np.asarray(inputs["inp_scales"], dtype=np.float32)
    zp_in = np.asarray(inputs["inp_zero_points"])
    s_w = np.asarray(inputs["weight_scales"], dtype=np.float32)
    zp_w = np.asarray(inputs["weight_zero_points"])

    x_eff = (inp.astype(np.int32) - np.int32(zp_in.reshape(-1)[0])).astype(bf16)
    w_eff = (weight.astype(np.int32) - zp_w.reshape(-1)[:, None].astype(np.int32)).astype(bf16)

    # w_tiled[p, no, ni, ko, nn] = w_eff[no*FD + ni*P + nn, ko*P + p]
    w_tiled = np.ascontiguousarray(
        w_eff.reshape(NTO, NTI, P, KT, P).transpose(4, 0, 1, 3, 2)
    )
    scl = (s_in.reshape(-1)[0] * s_w).astype(np.float32)
    sclbia = np.ascontiguousarray(np.stack([scl, bias]))  # [2, N]

    in_maps = []
    for c in range(NCORES):
        # x_tiled[p, mc, ko, mm] = x_eff[c*ML + mc*FD + mm, ko*P + p]
        xc = x_eff[c * ML : (c + 1) * ML]
        x_tiled = np.ascontiguousarray(
            xc.reshape(MH, FD, KT, P).transpose(3, 0, 2, 1)
        )
        in_maps.append({"xt": x_tiled, "wt": w_tiled, "sclbia": sclbia})
    return in_maps


def _run(inputs, trace=False):
    from concourse.bass_utils import run_bass_kernel_spmd

    nc = _get_nc()
    in_maps = _prep_in_maps(inputs)
    res = run_bass_kernel_spmd(nc, in_maps, list(range(NCORES)), trace=trace)
    out = np.empty((M, N), dtype=np.float32)
    for c in range(NCORES):
        out[c * ML : (c + 1) * ML, :] = res.results[c]["outT"].T
    return out, res


def _run_in_subprocess(inputs) -> np.ndarray:
    """Re-run in a fresh process. The axon/NRT first execution occasionally
    dies with NRT_EXEC_UNIT_UNRECOVERABLE and the in-process jax client is
    not reliably usable afterwards; a fresh process recovers."""
    import os
    import subprocess
    import sys
    import tempfile

    tmp = tempfile.mkdtemp(prefix="qlin_")
    in_path = os.path.join(tmp, "in.npz")
    out_path = os.path.join(tmp, "out.npy")
    np.savez(in_path, **{k: np.asarray(v) for k, v in inputs.items()})
    code = (
        "import importlib.util, numpy as np\n"
        f"spec = importlib.util.spec_from_file_location('qlin_kernel', {os.path.abspath(__file__)!r})\n"
        "m = importlib.util.module_from_spec(spec)\n"
        "spec.loader.exec_module(m)\n"
        f"data = np.load({in_path!r})\n"
        "inputs = {k: data[k] for k in data.files}\n"
        "out, _ = m._run(inputs, trace=False)\n"
        f"np.save({out_path!r}, out)\n"
    )
    env = dict(os.environ, QLIN_NO_SUBPROC="1")
    last_err = None
    for _ in range(2):
        try:
            subprocess.run(
                [sys.executable, "-c", code], env=env, check=True, timeout=1800
            )
            return np.load(out_path)
        except Exception as e:  # noqa: BLE001 - retry then re-raise
            last_err = e
    raise last_err


def kernel(**inputs) -> np.ndarray:
    import os

    try:
        out, _ = _run(inputs, trace=False)
        return out
    except Exception:
        if os.environ.get("QLIN_NO_SUBPROC"):
            raise
    return _run_in_subprocess(inputs)


# revision 22
# speedup vs baseline: 1.0226x; 1.0137x over previous
"""Qlinear (dequant int8 matmul + bias) Trainium2 kernel.

Reference computation (zero points subtracted exactly; per-output-channel
scales):
    out[m, n] = (sum_k (inp[m,k]-izp) * (w[n,k]-wzp[n])) * (is * ws[n]) + bias[n]

Strategy: int8-range values are exactly representable in bf16, and the PE
accumulates bf16 matmuls into fp32 PSUM, so the integer matmul is exact on
the TensorEngine.  The per-channel scale+bias is applied on PSUM eviction
with N on the partition axis (per-partition scale/bias on the ScalarEngine).

Sharding: M (rows of inp) is sharded across the 8 cores; each core keeps its
x-shard resident in SBUF (8MB bf16) and streams the full transposed weight
(33.5MB bf16) through SBUF in [K, 512] tiles.

Host pre-tiles both operands into the SBUF layout ([partition, ...tile dims])
so every DMA moves 8-32KB contiguous runs per partition (full HBM bandwidth);
the load order matches consumption order so the PE starts ~16us in and
DMA-chases the head instead of waiting for the full 12MB input fill.
"""

import numpy as np
import ml_dtypes

M, K, N = 8192, 4096, 4096
NCORES = 8
ML = M // NCORES  # 1024 rows of inp per core
P = 128
KT = K // P  # 32 k-tiles
FD = 512  # matmul moving free dim / PSUM bank width (fp32)
NTO = N // FD  # 8 outer n-tiles (one wsb load each)
NTI = FD // P  # 4 inner n-tiles (psum partition dim)
MH = ML // FD  # 2 psum groups over local M

_CACHE: dict = {}


def build_nc(ml=ML, k=K, n=N):
    import concourse.mybir as mybir
    import concourse.tile as tile
    from concourse import bacc

    kt = k // P
    nto = n // FD
    nti = FD // P
    mh = ml // FD

    nc = bacc.Bacc(None, target_bir_lowering=False)
    xt = nc.dram_tensor("xt", [P, mh, kt, FD], mybir.dt.bfloat16, kind="ExternalInput")
    wt = nc.dram_tensor(
        "wt", [P, nto, nti, kt, P], mybir.dt.bfloat16, kind="ExternalInput"
    )
    sclbia = nc.dram_tensor("sclbia", [2, n], mybir.dt.float32, kind="ExternalInput")
    outT = nc.dram_tensor("outT", [n, ml], mybir.dt.float32, kind="ExternalOutput")

    sclbia3 = sclbia.rearrange("t (nt p) -> p t nt", p=P)
    outT3 = outT.rearrange("(nt p) m -> p nt m", p=P)

    with tile.TileContext(nc) as tc:
        with (
            tc.tile_pool(name="const", bufs=1) as const,
            tc.tile_pool(name="xpool", bufs=1) as xpool,
            tc.tile_pool(name="wpool", bufs=2) as wpool,
            tc.tile_pool(name="opool", bufs=4) as opool,
            tc.tile_pool(name="psum", bufs=8, space="PSUM") as pspool,
        ):
            # Load order = consumption order: w[no=0,ni=0], x[mc=0], x[mc=1],
            # rest of w[no=0], consts, then w[no] streamed per outer n-tile.
            wsb0 = wpool.tile([P, nti, kt, P], mybir.dt.bfloat16, tag="wsb")
            nc.sync.dma_start(wsb0[:, 0], wt[:, 0, 0])
            xsb = xpool.tile([P, mh, kt, FD], mybir.dt.bfloat16)
            kh = kt // 2
            nc.sync.dma_start(xsb[:, 0, :kh], xt[:, 0, :kh])
            nc.sync.dma_start(xsb[:, 0, kh:], xt[:, 0, kh:])
            for ni in range(1, nti):
                nc.sync.dma_start(wsb0[:, ni], wt[:, 0, ni])
            for mc in range(1, mh):
                nc.sync.dma_start(xsb[:, mc], xt[:, mc])

            # Stage the per-channel scale/bias (not needed until the first
            # eviction), then copy via the ScalarE so the per-tile evictions
            # (also on ScalarE) depend on it through engine program order
            # (engine instructions carry few sync waits).
            csb_stage = const.tile([P, 2, n // P], mybir.dt.float32)
            nc.scalar.dma_start(csb_stage[:], sclbia3[:])
            csb = const.tile([P, 2, n // P], mybir.dt.float32)
            nc.scalar.copy(csb[:], csb_stage[:])
            scl_sb = csb[:, 0]
            bia_sb = csb[:, 1]

            for no in range(nto):
                if no == 0:
                    wsb = wsb0
                else:
                    wsb = wpool.tile([P, nti, kt, P], mybir.dt.bfloat16, tag="wsb")
                    nc.sync.dma_start(wsb[:], wt[:, no])
                # mc outer: the first nti psum groups only need x[mc=0], so
                # x[mc=1] arrives behind them during the head fill.
                for mc in range(mh):
                    for ni in range(nti):
                        n_abs = no * nti + ni
                        ps = pspool.tile([P, FD], mybir.dt.float32)
                        for ko in range(kt):
                            nc.tensor.matmul(
                                ps[:],
                                wsb[:, ni, ko, :],
                                xsb[:, mc, ko, :],
                                start=(ko == 0),
                                stop=(ko == kt - 1),
                            )
                        osb = opool.tile([P, FD], mybir.dt.float32)
                        nc.scalar.activation(
                            osb[:],
                            ps[:],
                            mybir.ActivationFunctionType.Identity,
                            bias=bia_sb[:, n_abs : n_abs + 1],
                            scale=scl_sb[:, n_abs : n_abs + 1],
                        )
                        nc.sync.dma_start(
                            outT3[:, n_abs, mc * FD : (mc + 1) * FD], osb[:]
                        )
    nc.compile()
    return nc


def _get_nc():
    if "nc" not in _CACHE:
        _CACHE["nc"] = build_nc()
    return _CACHE["nc"]


def _prep_in_maps(inputs):
    bf16 = ml_dtypes.bfloat16
    inp = np.asarray(inputs["inp"])
    weight = np.asarray(inputs["weight"])
    bias = np.ascontiguousarray(np.asarray(inputs["bias"], dtype=np.float32))
    s_in = np.asarray(inputs["inp_scales"], dtype=np.float32)
    zp_in = np.asarray(inputs["inp_zero_points"])
    s_w = np.asarray(inputs["weight_scales"], dtype=np.float32)
    zp_w = np.asarray(inputs["weight_zero_points"])

    x_eff = (inp.astype(np.int32) - np.int32(zp_in.reshape(-1)[0])).astype(bf16)
    w_eff = (weight.astype(np.int32) - zp_w.reshape(-1)[:, None].astype(np.int32)).astype(bf16)

    # w_tiled[p, no, ni, ko, nn] = w_eff[no*FD + ni*P + nn, ko*P + p]
    w_tiled = np.ascontiguousarray(
        w_eff.reshape(NTO, NTI, P, KT, P).transpose(4, 0, 1, 3, 2)
    )
    scl = (s_in.reshape(-1)[0] * s_w).astype(np.float32)
    sclbia = np.ascontiguousarray(np.stack([scl, bias]))  # [2, N]

    in_maps = []
    for c in range(NCORES):
        # x_tiled[p, mc, ko, mm] = x_eff[c*ML + mc*FD + mm, ko*P + p]
        xc = x_eff[c * ML : (c + 1) * ML]
        x_tiled = np.ascontiguousarray(
            xc.reshape(MH, FD, KT, P).transpose(3, 0, 2, 1)
        )
        in_maps.append({"xt": x_tiled, "wt": w_tiled, "sclbia": sclbia})
    return in_maps


def _run(inputs, trace=False):
    from concourse.bass_utils import run_bass_kernel_spmd

    nc = _get_nc()
    in_maps = _prep_in_maps(inputs)
    res = run_bass_kernel_spmd(nc, in_maps, list(range(NCORES)), trace=trace)
    out = np.empty((M, N), dtype=np.float32)
    for c in range(NCORES):
        out[c * ML : (c + 1) * ML, :] = res.results[c]["outT"].T
    return out, res


def _run_in_subprocess(inputs) -> np.ndarray:
    """Re-run in a fresh process. The axon/NRT first execution occasionally
    dies with NRT_EXEC_UNIT_UNRECOVERABLE and the in-process jax client is
    not reliably usable afterwards; a fresh process recovers."""
    import os
    import subprocess
    import sys
    import tempfile

    tmp = tempfile.mkdtemp(prefix="qlin_")
    in_path = os.path.join(tmp, "in.npz")
    out_path = os.path.join(tmp, "out.npy")
    np.savez(in_path, **{k: np.asarray(v) for k, v in inputs.items()})
    code = (
        "import importlib.util, numpy as np\n"
        f"spec = importlib.util.spec_from_file_location('qlin_kernel', {os.path.abspath(__file__)!r})\n"
        "m = importlib.util.module_from_spec(spec)\n"
        "spec.loader.exec_module(m)\n"
        f"data = np.load({in_path!r})\n"
        "inputs = {k: data[k] for k in data.files}\n"
        "out, _ = m._run(inputs, trace=False)\n"
        f"np.save({out_path!r}, out)\n"
    )
    env = dict(os.environ, QLIN_NO_SUBPROC="1")
    last_err = None
    for _ in range(2):
        try:
            subprocess.run(
                [sys.executable, "-c", code], env=env, check=True, timeout=1800
            )
            return np.load(out_path)
        except Exception as e:  # noqa: BLE001 - retry then re-raise
            last_err = e
    raise last_err


def kernel(**inputs) -> np.ndarray:
    import os

    try:
        out, _ = _run(inputs, trace=False)
        return out
    except Exception:
        if os.environ.get("QLIN_NO_SUBPROC"):
            raise
    return _run_in_subprocess(inputs)


# revision 23
# speedup vs baseline: 1.0283x; 1.0056x over previous
"""Qlinear (dequant int8 matmul + bias) Trainium2 kernel.

Reference computation (zero points subtracted exactly; per-output-channel
scales):
    out[m, n] = (sum_k (inp[m,k]-izp) * (w[n,k]-wzp[n])) * (is * ws[n]) + bias[n]

Strategy: int8-range values are exactly representable in bf16, and the PE
accumulates bf16 matmuls into fp32 PSUM, so the integer matmul is exact on
the TensorEngine.  The per-channel scale+bias is applied on PSUM eviction
with N on the partition axis (per-partition scale/bias on the ScalarEngine).

Sharding: M (rows of inp) is sharded across the 8 cores; each core keeps its
x-shard resident in SBUF (8MB bf16) and streams the full transposed weight
(33.5MB bf16) through SBUF in [K, 512] tiles.

Host pre-tiles both operands into the SBUF layout ([partition, ...tile dims])
so every DMA moves 8-32KB contiguous runs per partition (full HBM bandwidth);
the load order matches consumption order so the PE starts ~16us in and
DMA-chases the head instead of waiting for the full 12MB input fill.
"""

import numpy as np
import ml_dtypes

M, K, N = 8192, 4096, 4096
NCORES = 8
ML = M // NCORES  # 1024 rows of inp per core
P = 128
KT = K // P  # 32 k-tiles
FD = 512  # matmul moving free dim / PSUM bank width (fp32)
NTO = N // FD  # 8 outer n-tiles (one wsb load each)
NTI = FD // P  # 4 inner n-tiles (psum partition dim)
MH = ML // FD  # 2 psum groups over local M

_CACHE: dict = {}


def build_nc(ml=ML, k=K, n=N):
    import concourse.mybir as mybir
    import concourse.tile as tile
    from concourse import bacc

    kt = k // P
    nto = n // FD
    nti = FD // P
    mh = ml // FD

    nc = bacc.Bacc(None, target_bir_lowering=False)
    xt = nc.dram_tensor("xt", [P, mh, kt, FD], mybir.dt.bfloat16, kind="ExternalInput")
    wt = nc.dram_tensor(
        "wt", [P, nto, nti, kt, P], mybir.dt.bfloat16, kind="ExternalInput"
    )
    sclbia = nc.dram_tensor("sclbia", [2, n], mybir.dt.float32, kind="ExternalInput")
    outT = nc.dram_tensor("outT", [n, ml], mybir.dt.float32, kind="ExternalOutput")

    sclbia3 = sclbia.rearrange("t (nt p) -> p t nt", p=P)
    outT3 = outT.rearrange("(nt p) m -> p nt m", p=P)

    with tile.TileContext(nc) as tc:
        with (
            tc.tile_pool(name="const", bufs=1) as const,
            tc.tile_pool(name="xpool", bufs=1) as xpool,
            tc.tile_pool(name="wpool", bufs=2) as wpool,
            tc.tile_pool(name="opool", bufs=4) as opool,
            tc.tile_pool(name="psum", bufs=8, space="PSUM") as pspool,
        ):
            # Load order = consumption order: w[no=0,ni=0], x[mc=0], x[mc=1],
            # rest of w[no=0], consts, then w[no] streamed per outer n-tile.
            wsb0 = wpool.tile([P, nti, kt, P], mybir.dt.bfloat16, tag="wsb")
            nc.sync.dma_start(wsb0[:, 0], wt[:, 0, 0])
            xsb = xpool.tile([P, mh, kt, FD], mybir.dt.bfloat16)
            kh = kt // 2
            nc.sync.dma_start(xsb[:, 0, :kh], xt[:, 0, :kh])
            nc.sync.dma_start(xsb[:, 0, kh:], xt[:, 0, kh:])
            for ni in range(1, nti):
                nc.sync.dma_start(wsb0[:, ni], wt[:, 0, ni])
            for mc in range(1, mh):
                nc.sync.dma_start(xsb[:, mc], xt[:, mc])

            # Warm the PE clock gate (HAM) while the head DMAs stream: ~16
            # dummy matmuls on zeroed scratch keep the PE busy from ~6us so
            # the real matmuls at ~14us start at 2.4GHz instead of 1.2GHz.
            warmw = const.tile([P, P], mybir.dt.bfloat16)
            nc.gpsimd.memset(warmw[:], 0.0)
            warmx = const.tile([P, FD], mybir.dt.bfloat16)
            nc.gpsimd.memset(warmx[:], 0.0)
            warmps = pspool.tile([P, FD], mybir.dt.float32, tag="ps")
            for _ in range(16):
                nc.tensor.matmul(warmps[:], warmw[:], warmx[:], start=True, stop=True)

            # Stage the per-channel scale/bias (not needed until the first
            # eviction), then copy via the ScalarE so the per-tile evictions
            # (also on ScalarE) depend on it through engine program order
            # (engine instructions carry few sync waits).
            csb_stage = const.tile([P, 2, n // P], mybir.dt.float32)
            nc.scalar.dma_start(csb_stage[:], sclbia3[:])
            csb = const.tile([P, 2, n // P], mybir.dt.float32)
            nc.scalar.copy(csb[:], csb_stage[:])
            scl_sb = csb[:, 0]
            bia_sb = csb[:, 1]

            for no in range(nto):
                if no == 0:
                    wsb = wsb0
                else:
                    wsb = wpool.tile([P, nti, kt, P], mybir.dt.bfloat16, tag="wsb")
                    nc.sync.dma_start(wsb[:], wt[:, no])
                # mc outer: the first nti psum groups only need x[mc=0], so
                # x[mc=1] arrives behind them during the head fill.
                for mc in range(mh):
                    for ni in range(nti):
                        n_abs = no * nti + ni
                        ps = pspool.tile([P, FD], mybir.dt.float32)
                        for ko in range(kt):
                            nc.tensor.matmul(
                                ps[:],
                                wsb[:, ni, ko, :],
                                xsb[:, mc, ko, :],
                                start=(ko == 0),
                                stop=(ko == kt - 1),
                            )
                        osb = opool.tile([P, FD], mybir.dt.float32)
                        nc.scalar.activation(
                            osb[:],
                            ps[:],
                            mybir.ActivationFunctionType.Identity,
                            bias=bia_sb[:, n_abs : n_abs + 1],
                            scale=scl_sb[:, n_abs : n_abs + 1],
                        )
                        nc.sync.dma_start(
                            outT3[:, n_abs, mc * FD : (mc + 1) * FD], osb[:]
                        )
    nc.compile()
    return nc


def _get_nc():
    if "nc" not in _CACHE:
        _CACHE["nc"] = build_nc()
    return _CACHE["nc"]


def _prep_in_maps(inputs):
    bf16 = ml_dtypes.bfloat16
    inp = np.asarray(inputs["inp"])
    weight = np.asarray(inputs["weight"])
    bias = np.ascontiguousarray(np.asarray(inputs["bias"], dtype=np.float32))
    s_in = np.asarray(inputs["inp_scales"], dtype=np.float32)
    zp_in = np.asarray(inputs["inp_zero_points"])
    s_w = np.asarray(inputs["weight_scales"], dtype=np.float32)
    zp_w = np.asarray(inputs["weight_zero_points"])

    x_eff = (inp.astype(np.int32) - np.int32(zp_in.reshape(-1)[0])).astype(bf16)
    w_eff = (weight.astype(np.int32) - zp_w.reshape(-1)[:, None].astype(np.int32)).astype(bf16)

    # w_tiled[p, no, ni, ko, nn] = w_eff[no*FD + ni*P + nn, ko*P + p]
    w_tiled = np.ascontiguousarray(
        w_eff.reshape(NTO, NTI, P, KT, P).transpose(4, 0, 1, 3, 2)
    )
    scl = (s_in.reshape(-1)[0] * s_w).astype(np.float32)
    sclbia = np.ascontiguousarray(np.stack([scl, bias]))  # [2, N]

    in_maps = []
    for c in range(NCORES):
        # x_tiled[p, mc, ko, mm] = x_eff[c*ML + mc*FD + mm, ko*P + p]
        xc = x_eff[c * ML : (c + 1) * ML]
        x_tiled = np.ascontiguousarray(
            xc.reshape(MH, FD, KT, P).transpose(3, 0, 2, 1)
        )
        in_maps.append({"xt": x_tiled, "wt": w_tiled, "sclbia": sclbia})
    return in_maps


def _run(inputs, trace=False):
    from concourse.bass_utils import run_bass_kernel_spmd

    nc = _get_nc()
    in_maps = _prep_in_maps(inputs)
    res = run_bass_kernel_spmd(nc, in_maps, list(range(NCORES)), trace=trace)
    out = np.empty((M, N), dtype=np.float32)
    for c in range(NCORES):
        out[c * ML : (c + 1) * ML, :] = res.results[c]["outT"].T
    return out, res


def _run_in_subprocess(inputs) -> np.ndarray:
    """Re-run in a fresh process. The axon/NRT first execution occasionally
    dies with NRT_EXEC_UNIT_UNRECOVERABLE and the in-process jax client is
    not reliably usable afterwards; a fresh process recovers."""
    import os
    import subprocess
    import sys
    import tempfile

    tmp = tempfile.mkdtemp(prefix="qlin_")
    in_path = os.path.join(tmp, "in.npz")
    out_path = os.path.join(tmp, "out.npy")
    np.savez(in_path, **{k: np.asarray(v) for k, v in inputs.items()})
    code = (
        "import importlib.util, numpy as np\n"
        f"spec = importlib.util.spec_from_file_location('qlin_kernel', {os.path.abspath(__file__)!r})\n"
        "m = importlib.util.module_from_spec(spec)\n"
        "spec.loader.exec_module(m)\n"
        f"data = np.load({in_path!r})\n"
        "inputs = {k: data[k] for k in data.files}\n"
        "out, _ = m._run(inputs, trace=False)\n"
        f"np.save({out_path!r}, out)\n"
    )
    env = dict(os.environ, QLIN_NO_SUBPROC="1")
    last_err = None
    for _ in range(2):
        try:
            subprocess.run(
                [sys.executable, "-c", code], env=env, check=True, timeout=1800
            )
            return np.load(out_path)
        except Exception as e:  # noqa: BLE001 - retry then re-raise
            last_err = e
    raise last_err


def kernel(**inputs) -> np.ndarray:
    import os

    try:
        out, _ = _run(inputs, trace=False)
        return out
    except Exception:
        if os.environ.get("QLIN_NO_SUBPROC"):
            raise
    return _run_in_subprocess(inputs)


# revision 24
# speedup vs baseline: 1.0287x; 1.0004x over previous
"""Qlinear (dequant int8 matmul + bias) Trainium2 kernel.

Reference computation (zero points subtracted exactly; per-output-channel
scales):
    out[m, n] = (sum_k (inp[m,k]-izp) * (w[n,k]-wzp[n])) * (is * ws[n]) + bias[n]

Strategy: int8-range values are exactly representable in bf16, and the PE
accumulates bf16 matmuls into fp32 PSUM, so the integer matmul is exact on
the TensorEngine.  The per-channel scale+bias is applied on PSUM eviction
with N on the partition axis (per-partition scale/bias on the ScalarEngine).

Sharding: M (rows of inp) is sharded across the 8 cores; each core keeps its
x-shard resident in SBUF (8MB bf16) and streams the full transposed weight
(33.5MB bf16) through SBUF in [K, 512] tiles.

Host pre-tiles both operands into the SBUF layout ([partition, ...tile dims])
so every DMA moves 8-32KB contiguous runs per partition (full HBM bandwidth);
the load order matches consumption order so the PE starts ~16us in and
DMA-chases the head instead of waiting for the full 12MB input fill.
"""

import numpy as np
import ml_dtypes

M, K, N = 8192, 4096, 4096
NCORES = 8
ML = M // NCORES  # 1024 rows of inp per core
P = 128
KT = K // P  # 32 k-tiles
FD = 512  # matmul moving free dim / PSUM bank width (fp32)
NTO = N // FD  # 8 outer n-tiles (one wsb load each)
NTI = FD // P  # 4 inner n-tiles (psum partition dim)
MH = ML // FD  # 2 psum groups over local M

_CACHE: dict = {}


def build_nc(ml=ML, k=K, n=N):
    import concourse.mybir as mybir
    import concourse.tile as tile
    from concourse import bacc

    kt = k // P
    nto = n // FD
    nti = FD // P
    mh = ml // FD

    nc = bacc.Bacc(None, target_bir_lowering=False)
    xt = nc.dram_tensor("xt", [P, mh, kt, FD], mybir.dt.bfloat16, kind="ExternalInput")
    wt = nc.dram_tensor(
        "wt", [P, nto, nti, kt, P], mybir.dt.bfloat16, kind="ExternalInput"
    )
    sclbia = nc.dram_tensor("sclbia", [2, n], mybir.dt.float32, kind="ExternalInput")
    outT = nc.dram_tensor("outT", [n, ml], mybir.dt.float32, kind="ExternalOutput")

    sclbia3 = sclbia.rearrange("t (nt p) -> p t nt", p=P)
    outT3 = outT.rearrange("(nt p) m -> p nt m", p=P)

    with tile.TileContext(nc) as tc:
        with (
            tc.tile_pool(name="const", bufs=1) as const,
            tc.tile_pool(name="xpool", bufs=1) as xpool,
            tc.tile_pool(name="wpool", bufs=2) as wpool,
            tc.tile_pool(name="opool", bufs=4) as opool,
            tc.tile_pool(name="psum", bufs=8, space="PSUM") as pspool,
        ):
            # Load order = consumption order: w[no=0,ni=0], x[mc=0], x[mc=1],
            # rest of w[no=0], consts, then w[no] streamed per outer n-tile.
            wsb0 = wpool.tile([P, nti, kt, P], mybir.dt.bfloat16, tag="wsb")
            nc.sync.dma_start(wsb0[:, 0], wt[:, 0, 0])
            xsb = xpool.tile([P, mh, kt, FD], mybir.dt.bfloat16)
            kh = kt // 2
            nc.sync.dma_start(xsb[:, 0, :kh], xt[:, 0, :kh])
            nc.sync.dma_start(xsb[:, 0, kh:], xt[:, 0, kh:])
            for ni in range(1, nti):
                nc.sync.dma_start(wsb0[:, ni], wt[:, 0, ni])
            for mc in range(1, mh):
                nc.sync.dma_start(xsb[:, mc], xt[:, mc])

            # Warm the PE clock gate (HAM) while the head DMAs stream: ~16
            # dummy matmuls on zeroed scratch keep the PE busy from ~6us so
            # the real matmuls at ~14us start at 2.4GHz instead of 1.2GHz.
            warmw = const.tile([P, P], mybir.dt.bfloat16)
            nc.gpsimd.memset(warmw[:], 0.0)
            warmx = const.tile([P, FD], mybir.dt.bfloat16)
            nc.gpsimd.memset(warmx[:], 0.0)
            warmps = pspool.tile([P, FD], mybir.dt.float32, tag="ps")
            for _ in range(36):
                nc.tensor.matmul(warmps[:], warmw[:], warmx[:], start=True, stop=True)

            # Stage the per-channel scale/bias (not needed until the first
            # eviction), then copy via the ScalarE so the per-tile evictions
            # (also on ScalarE) depend on it through engine program order
            # (engine instructions carry few sync waits).
            csb_stage = const.tile([P, 2, n // P], mybir.dt.float32)
            nc.scalar.dma_start(csb_stage[:], sclbia3[:])
            csb = const.tile([P, 2, n // P], mybir.dt.float32)
            nc.scalar.copy(csb[:], csb_stage[:])
            scl_sb = csb[:, 0]
            bia_sb = csb[:, 1]

            for no in range(nto):
                if no == 0:
                    wsb = wsb0
                else:
                    wsb = wpool.tile([P, nti, kt, P], mybir.dt.bfloat16, tag="wsb")
                    nc.sync.dma_start(wsb[:], wt[:, no])
                # mc outer: the first nti psum groups only need x[mc=0], so
                # x[mc=1] arrives behind them during the head fill.
                for mc in range(mh):
                    for ni in range(nti):
                        n_abs = no * nti + ni
                        ps = pspool.tile([P, FD], mybir.dt.float32)
                        for ko in range(kt):
                            nc.tensor.matmul(
                                ps[:],
                                wsb[:, ni, ko, :],
                                xsb[:, mc, ko, :],
                                start=(ko == 0),
                                stop=(ko == kt - 1),
                            )
                        osb = opool.tile([P, FD], mybir.dt.float32)
                        nc.scalar.activation(
                            osb[:],
                            ps[:],
                            mybir.ActivationFunctionType.Identity,
                            bias=bia_sb[:, n_abs : n_abs + 1],
                            scale=scl_sb[:, n_abs : n_abs + 1],
                        )
                        nc.sync.dma_start(
                            outT3[:, n_abs, mc * FD : (mc + 1) * FD], osb[:]
                        )
    nc.compile()
    return nc


def _get_nc():
    if "nc" not in _CACHE:
        _CACHE["nc"] = build_nc()
    return _CACHE["nc"]


def _prep_in_maps(inputs):
    bf16 = ml_dtypes.bfloat16
    inp = np.asarray(inputs["inp"])
    weight = np.asarray(inputs["weight"])
    bias = np.ascontiguousarray(np.asarray(inputs["bias"], dtype=np.float32))
    s_in = np.asarray(inputs["inp_scales"], dtype=np.float32)
    zp_in = np.asarray(inputs["inp_zero_points"])
    s_w = np.asarray(inputs["weight_scales"], dtype=np.float32)
    zp_w = np.asarray(inputs["weight_zero_points"])

    x_eff = (inp.astype(np.int32) - np.int32(zp_in.reshape(-1)[0])).astype(bf16)
    w_eff = (weight.astype(np.int32) - zp_w.reshape(-1)[:, None].astype(np.int32)).astype(bf16)

    # w_tiled[p, no, ni, ko, nn] = w_eff[no*FD + ni*P + nn, ko*P + p]
    w_tiled = np.ascontiguousarray(
        w_eff.reshape(NTO, NTI, P, KT, P).transpose(4, 0, 1, 3, 2)
    )
    scl = (s_in.reshape(-1)[0] * s_w).astype(np.float32)
    sclbia = np.ascontiguousarray(np.stack([scl, bias]))  # [2, N]

    in_maps = []
    for c in range(NCORES):
        # x_tiled[p, mc, ko, mm] = x_eff[c*ML + mc*FD + mm, ko*P + p]
        xc = x_eff[c * ML : (c + 1) * ML]
        x_tiled = np.ascontiguousarray(
            xc.reshape(MH, FD, KT, P).transpose(3, 0, 2, 1)
        )
        in_maps.append({"xt": x_tiled, "wt": w_tiled, "sclbia": sclbia})
    return in_maps


def _run(inputs, trace=False):
    from concourse.bass_utils import run_bass_kernel_spmd

    nc = _get_nc()
    in_maps = _prep_in_maps(inputs)
    res = run_bass_kernel_spmd(nc, in_maps, list(range(NCORES)), trace=trace)
    out = np.empty((M, N), dtype=np.float32)
    for c in range(NCORES):
        out[c * ML : (c + 1) * ML, :] = res.results[c]["outT"].T
    return out, res


def _run_in_subprocess(inputs) -> np.ndarray:
    """Re-run in a fresh process. The axon/NRT first execution occasionally
    dies with NRT_EXEC_UNIT_UNRECOVERABLE and the in-process jax client is
    not reliably usable afterwards; a fresh process recovers."""
    import os
    import subprocess
    import sys
    import tempfile

    tmp = tempfile.mkdtemp(prefix="qlin_")
    in_path = os.path.join(tmp, "in.npz")
    out_path = os.path.join(tmp, "out.npy")
    np.savez(in_path, **{k: np.asarray(v) for k, v in inputs.items()})
    code = (
        "import importlib.util, numpy as np\n"
        f"spec = importlib.util.spec_from_file_location('qlin_kernel', {os.path.abspath(__file__)!r})\n"
        "m = importlib.util.module_from_spec(spec)\n"
        "spec.loader.exec_module(m)\n"
        f"data = np.load({in_path!r})\n"
        "inputs = {k: data[k] for k in data.files}\n"
        "out, _ = m._run(inputs, trace=False)\n"
        f"np.save({out_path!r}, out)\n"
    )
    env = dict(os.environ, QLIN_NO_SUBPROC="1")
    last_err = None
    for _ in range(2):
        try:
            subprocess.run(
                [sys.executable, "-c", code], env=env, check=True, timeout=1800
            )
            return np.load(out_path)
        except Exception as e:  # noqa: BLE001 - retry then re-raise
            last_err = e
    raise last_err


def kernel(**inputs) -> np.ndarray:
    import os

    try:
        out, _ = _run(inputs, trace=False)
        return out
    except Exception:
        if os.environ.get("QLIN_NO_SUBPROC"):
            raise
    return _run_in_subprocess(inputs)
